# revision 1
# baseline (speedup 1.0000x reference)
"""Trainium2 Bass kernel for the LogicLayer (difflogic) problem, v2.

out[i, o] = c0[o] + ca[o]*a + cb[o]*b + cab[o]*a*b
  with a = x[i, idx_a[o]], b = x[i, idx_b[o]],
  [c0, ca, cb, cab] = softmax(weights[o]) @ GATE_COEFFS.

Strategy (8 cores, OUTPUT-sharded, output-major, fp16):
  - host pre-transposes x to xT [IN_DIM, BATCH] fp16; each core owns
    OSHARD = OUT_DIM/8 = 1024 output columns.
  - per 128-output chunk, gpsimd.dma_gather pulls the needed xT rows
    straight from HBM into SBUF: a_tile[p, :] = xT[idx_a[g*128+p], :]
    (one 8 KiB descriptor per index; partition p <- index p).
  - coefficients become per-partition scalars [128,1]:
      DVE : s = a*cab + cb        (fused tensor_scalar, 4x mode)
      ACT : r = a*ca + c0         (activation Identity, scale+bias)
      DVE : s = s*b ; out = s + r (tensor_tensor, 2x mode)
  - y stored output-major [128, G, BATCH] fp16; host unpermutes.
"""

import numpy as np

BATCH, IN_DIM, OUT_DIM = 4096, 8192, 8192
N_CORES = 8
OSHARD = OUT_DIM // N_CORES     # 1024 outputs per core
P = 128
G = OSHARD // P                 # 8 chunks of 128 outputs
DEPTH = 3                       # gather/out tile pipeline depth

GATE_COEFFS = np.array([
    [0, 0, 0, 0], [0, 0, 0, 1], [0, 1, 0, -1], [0, 1, 0, 0],
    [0, 0, 1, -1], [0, 0, 1, 0], [0, 1, 1, -2], [0, 1, 1, -1],
    [1, -1, -1, 1], [1, -1, -1, 2], [1, 0, -1, 0], [1, 0, -1, 1],
    [1, -1, 0, 0], [1, -1, 0, 1], [1, 0, 0, -1], [1, 0, 0, 0],
], dtype=np.float32)  # [16, 4]

_CACHE = {}


def _build_nc(n_reps=1):
    import concourse.bacc as bacc
    import concourse.bass as bass
    import concourse.mybir as mybir
    from concourse.library_config import mlp

    f16 = mybir.dt.float16
    f32 = mybir.dt.float32
    i16 = mybir.dt.int16
    Alu = mybir.AluOpType
    Act = mybir.ActivationFunctionType

    nc = bacc.Bacc("TRN2", target_bir_lowering=False, debug=False,
                   num_devices=N_CORES)
    xt = nc.dram_tensor("xt", [IN_DIM, BATCH], f16, kind="ExternalInput")
    idxw = nc.dram_tensor("idxw", [P, 2 * G * 8], i16, kind="ExternalInput")
    cf = nc.dram_tensor("cf", [P, G, 4], f32, kind="ExternalInput")
    y = nc.dram_tensor("y", [P, G, BATCH], f16, kind="ExternalOutput")

    T = n_reps * G

    with nc.sbuf_tensor("idx_sb", [P, 2 * G * 8], i16) as idx_sb, \
         nc.sbuf_tensor("cf_sb", [P, G, 4], f32) as cf_sb, \
         nc.sbuf_tensor("ga", [P, DEPTH, 1, BATCH], f16) as ga, \
         nc.sbuf_tensor("gb", [P, DEPTH, 1, BATCH], f16) as gb, \
         nc.sbuf_tensor("rr", [P, DEPTH, BATCH], f16) as rr, \
         nc.sbuf_tensor("ss", [P, BATCH], f16) as ss, \
         nc.sbuf_tensor("oo", [P, DEPTH, BATCH], f16) as oo, \
         nc.semaphore("setup") as setup_sem, \
         nc.semaphore("gasem0") as ga0, \
         nc.semaphore("gasem1") as ga1, \
         nc.semaphore("gasem2") as ga2, \
         nc.semaphore("gbsem0") as gb0, \
         nc.semaphore("gbsem1") as gb1, \
         nc.semaphore("gbsem2") as gb2, \
         nc.semaphore("ysem0") as y0, \
         nc.semaphore("ysem1") as y1, \
         nc.semaphore("ysem2") as y2, \
         nc.semaphore("actsem") as act_sem, \
         nc.semaphore("compsem") as comp_sem, \
         nc.semaphore("dvesem") as dve_sem, \
         nc.Block() as block:

        ga_sems = [ga0, ga1, ga2]
        gb_sems = [gb0, gb1, gb2]
        y_sems = [y0, y1, y2]

        @block.gpsimd
        def _(gp: bass.BassGpSimd):
            gp.load_library(mlp)
            gp.dma_start(idx_sb[:], idxw[:]).then_inc(setup_sem, 16)
            gp.dma_start(cf_sb[:], cf[:]).then_inc(setup_sem, 16)
            gp.wait_ge(setup_sem, 32)
            for t in range(T):
                g = t % G
                slot = t % DEPTH
                k = t // DEPTH
                if t >= DEPTH:
                    gp.wait_ge(comp_sem, t - DEPTH + 1)
                    gp.wait_ge(ga_sems[slot], 16 * k)
                    gp.wait_ge(gb_sems[slot], 16 * k)
                gp.dma_gather(
                    ga[:, slot, :, :], xt[:, :], idx_sb[:, g * 8:(g + 1) * 8],
                    P, P, BATCH,
                ).then_inc(ga_sems[slot], 16)
                gp.dma_gather(
                    gb[:, slot, :, :], xt[:, :],
                    idx_sb[:, G * 8 + g * 8: G * 8 + (g + 1) * 8],
                    P, P, BATCH,
                ).then_inc(gb_sems[slot], 16)

        @block.scalar
        def _(act: bass.BassScalarEngine):
            act.wait_ge(setup_sem, 32)
            for t in range(T):
                g = t % G
                slot = t % DEPTH
                if t >= DEPTH:
                    act.wait_ge(comp_sem, t - DEPTH + 1)
                act.wait_ge(ga_sems[slot], 16 * (t // DEPTH + 1))
                act.activation(
                    rr[:, slot, :], ga[:, slot, 0, :], Act.Identity,
                    bias=cf_sb[:, g, 0:1], scale=cf_sb[:, g, 1:2],
                ).then_inc(act_sem, 1)

        @block.vector
        def _(v: bass.BassVectorEngine):
            v.wait_ge(setup_sem, 32)
            for t in range(T):
                g = t % G
                slot = t % DEPTH
                v.wait_ge(ga_sems[slot], 16 * (t // DEPTH + 1))
                if t > 0:
                    v.wait_ge(comp_sem, t)  # ss WAR vs prior out-op read
                # s = a*cab + cb
                v.tensor_scalar(
                    ss[:], ga[:, slot, 0, :],
                    cf_sb[:, g, 3:4], cf_sb[:, g, 2:3],
                    Alu.mult, Alu.add,
                ).then_inc(dve_sem, 1)
                v.wait_ge(gb_sems[slot], 16 * (t // DEPTH + 1))
                v.wait_ge(dve_sem, 2 * t + 1)
                # s = s*b
                v.tensor_tensor(
                    ss[:], ss[:], gb[:, slot, 0, :], Alu.mult,
                ).then_inc(dve_sem, 1)
                v.wait_ge(act_sem, t + 1)
                v.wait_ge(dve_sem, 2 * t + 2)
                if t >= DEPTH:
                    v.wait_ge(y_sems[slot], 16 * (t // DEPTH))
                # out = s + r
                v.tensor_tensor(
                    oo[:, slot, :], ss[:], rr[:, slot, :], Alu.add,
                ).then_inc(comp_sem, 1)

        @block.sync
        def _(sp: bass.BassEngine):
            for t in range(T):
                g = t % G
                slot = t % DEPTH
                sp.wait_ge(comp_sem, t + 1)
                if t >= DEPTH:
                    sp.wait_ge(y_sems[slot], 16 * (t // DEPTH))
                sp.dma_start(y[:, g, :], oo[:, slot, :]).then_inc(
                    y_sems[slot], 16)
            for s_i in range(DEPTH):
                sp.wait_ge(y_sems[s_i], 16 * ((T - 1 - s_i) // DEPTH + 1))

    nc.compile()
    return nc


def _prep_host(x, weights, idx_a, idx_b):
    x = np.asarray(x, dtype=np.float32)
    w = np.asarray(weights, dtype=np.float32)
    e = np.exp(w - w.max(axis=1, keepdims=True))
    sm = e / e.sum(axis=1, keepdims=True)
    coeffs = (sm @ GATE_COEFFS).astype(np.float32)          # [8192, 4]
    xt = np.ascontiguousarray(x.T).astype(np.float16)       # [8192, 4096]
    ia = np.asarray(idx_a).astype(np.int16)
    ib = np.asarray(idx_b).astype(np.int16)

    def wrap(seq):  # [n] -> [128, n/16]: j at [j%16, j//16], tiled to 128
        m = seq.reshape(len(seq) // 16, 16).T
        return np.tile(m, (P // 16, 1))

    in_maps = []
    for c in range(N_CORES):
        sl = slice(c * OSHARD, (c + 1) * OSHARD)
        idxw = np.ascontiguousarray(
            np.concatenate([wrap(ia[sl]), wrap(ib[sl])], axis=1))  # [128,128]
        cfc = np.ascontiguousarray(
            coeffs[sl].reshape(G, P, 4).transpose(1, 0, 2))        # [128,8,4]
        in_maps.append({"xt": xt, "idxw": idxw, "cf": cfc})
    return in_maps


def _in_maps(x, weights, idx_a, idx_b):
    return _prep_host(x, weights, idx_a, idx_b)


def kernel(x, weights, idx_a, idx_b):
    from concourse.bass_utils import run_bass_kernel_spmd

    in_maps = _in_maps(x, weights, idx_a, idx_b)
    if "nc" not in _CACHE:
        _CACHE["nc"] = _build_nc()
    nc = _CACHE["nc"]
    res = run_bass_kernel_spmd(nc, in_maps, list(range(N_CORES)))
    outs = []
    for c in range(N_CORES):
        yc = res.results[c]["y"]                    # [128, G, 4096] fp16
        outs.append(yc.transpose(2, 1, 0).reshape(BATCH, OSHARD))
    return np.concatenate(outs, axis=1).astype(np.float32)



# revision 2
# speedup vs baseline: 1.3194x; 1.3194x over previous
"""Trainium2 Bass kernel for the LogicLayer (difflogic) problem, v3.

out[i, o] = c0[o] + ca[o]*a + cb[o]*b + cab[o]*a*b
  with a = x[i, idx_a[o]], b = x[i, idx_b[o]],
  [c0, ca, cb, cab] = softmax(weights[o]) @ GATE_COEFFS.

v3 strategy (8 cores, OUTPUT-sharded, output-major, fp16 in / uint8 out):
  - host pre-transposes x to xT [IN_DIM, BATCH] fp16; each core owns
    OSHARD = OUT_DIM/8 = 1024 output columns.
  - per 128-output chunk, gpsimd.dma_gather pulls the needed xT rows
    straight from HBM into SBUF (one 8 KiB descriptor per index).
  - DVE (all 4 passes):
      t = a*cab + cb    (tensor_scalar, 4x)
      r = a*ca  + c0    (tensor_scalar, 4x)
      u = t*b           (tensor_tensor, 2x)
      y16 = u + r       (tensor_tensor, 2x)
  - ACT: y8 = Identity(y16*253 + 1.25) cast to uint8 (free scale+bias).
  - y stored output-major [128, G, BATCH] uint8 (half the write bytes of
    fp16); host decodes y = (y8 - 1.0)/253 and unpermutes.
  HBM per core/rep: 16 MiB gather + 4 MiB write = 20 MiB (~55.6us floor)
  vs v2's 24 MiB (~66.7us).
"""

import numpy as np

BATCH, IN_DIM, OUT_DIM = 4096, 8192, 8192
N_CORES = 8
OSHARD = OUT_DIM // N_CORES     # 1024 outputs per core
P = 128
G = OSHARD // P                 # 8 chunks of 128 outputs
DEPTH = 3                       # gather/out tile pipeline depth

YSCALE = 253.0                  # y8 = round_or_floor(253*y + 1.25)
YBIAS_ENC = 1.25
YBIAS_DEC = 1.0

GATE_COEFFS = np.array([
    [0, 0, 0, 0], [0, 0, 0, 1], [0, 1, 0, -1], [0, 1, 0, 0],
    [0, 0, 1, -1], [0, 0, 1, 0], [0, 1, 1, -2], [0, 1, 1, -1],
    [1, -1, -1, 1], [1, -1, -1, 2], [1, 0, -1, 0], [1, 0, -1, 1],
    [1, -1, 0, 0], [1, -1, 0, 1], [1, 0, 0, -1], [1, 0, 0, 0],
], dtype=np.float32)  # [16, 4]

_CACHE = {}


def _build_nc(n_reps=1):
    import concourse.bacc as bacc
    import concourse.bass as bass
    import concourse.mybir as mybir
    from concourse.library_config import mlp

    f16 = mybir.dt.float16
    f32 = mybir.dt.float32
    i16 = mybir.dt.int16
    u8 = mybir.dt.uint8
    Alu = mybir.AluOpType
    Act = mybir.ActivationFunctionType

    nc = bacc.Bacc("TRN2", target_bir_lowering=False, debug=False,
                   num_devices=N_CORES)
    xt = nc.dram_tensor("xt", [IN_DIM, BATCH], f16, kind="ExternalInput")
    idxw = nc.dram_tensor("idxw", [P, 2 * G * 8], i16, kind="ExternalInput")
    cf = nc.dram_tensor("cf", [P, G, 4], f32, kind="ExternalInput")
    yc = nc.dram_tensor("yc", [P, 2], f32, kind="ExternalInput")
    y = nc.dram_tensor("y", [P, G, BATCH], u8, kind="ExternalOutput")

    T = n_reps * G

    from contextlib import ExitStack

    with ExitStack() as stack:
        ent = stack.enter_context
        idx_sb = ent(nc.sbuf_tensor("idx_sb", [P, 2 * G * 8], i16))
        cf_sb = ent(nc.sbuf_tensor("cf_sb", [P, G, 4], f32))
        yc_sb = ent(nc.sbuf_tensor("yc_sb", [P, 2], f32))
        ga = ent(nc.sbuf_tensor("ga", [P, DEPTH, 1, BATCH], f16))
        gb = ent(nc.sbuf_tensor("gb", [P, DEPTH, 1, BATCH], f16))
        tt = ent(nc.sbuf_tensor("tt", [P, BATCH], f16))
        rr = ent(nc.sbuf_tensor("rr", [P, BATCH], f16))
        uu = ent(nc.sbuf_tensor("uu", [P, BATCH], f16))
        y16 = ent(nc.sbuf_tensor("y16", [P, 2, BATCH], f16))
        y8 = ent(nc.sbuf_tensor("y8", [P, DEPTH, BATCH], u8))
        setup_sem = ent(nc.semaphore("setup"))
        ga0 = ent(nc.semaphore("gasem0"))
        ga1 = ent(nc.semaphore("gasem1"))
        ga2 = ent(nc.semaphore("gasem2"))
        gb0 = ent(nc.semaphore("gbsem0"))
        gb1 = ent(nc.semaphore("gbsem1"))
        gb2 = ent(nc.semaphore("gbsem2"))
        y0 = ent(nc.semaphore("ysem0"))
        y1 = ent(nc.semaphore("ysem1"))
        y2 = ent(nc.semaphore("ysem2"))
        act_sem = ent(nc.semaphore("actsem"))
        dve_sem = ent(nc.semaphore("dvesem"))
        block = ent(nc.Block())

        ga_sems = [ga0, ga1, ga2]
        gb_sems = [gb0, gb1, gb2]
        y_sems = [y0, y1, y2]

        @block.gpsimd
        def _(gp: bass.BassGpSimd):
            gp.load_library(mlp)
            gp.dma_start(idx_sb[:], idxw[:]).then_inc(setup_sem, 16)
            gp.dma_start(cf_sb[:], cf[:]).then_inc(setup_sem, 16)
            gp.dma_start(yc_sb[:], yc[:]).then_inc(setup_sem, 16)
            gp.wait_ge(setup_sem, 48)
            for t in range(T):
                g = t % G
                slot = t % DEPTH
                k = t // DEPTH
                if t >= DEPTH:
                    # chunk t-DEPTH fully consumed by DVE (y16 written)
                    gp.wait_ge(dve_sem, t - DEPTH + 1)
                    gp.wait_ge(ga_sems[slot], 16 * k)
                    gp.wait_ge(gb_sems[slot], 16 * k)
                gp.dma_gather(
                    ga[:, slot, :, :], xt[:, :], idx_sb[:, g * 8:(g + 1) * 8],
                    P, P, BATCH,
                ).then_inc(ga_sems[slot], 16)
                gp.dma_gather(
                    gb[:, slot, :, :], xt[:, :],
                    idx_sb[:, G * 8 + g * 8: G * 8 + (g + 1) * 8],
                    P, P, BATCH,
                ).then_inc(gb_sems[slot], 16)

        @block.vector
        def _(v: bass.BassVectorEngine):
            v.wait_ge(setup_sem, 48)
            for t in range(T):
                g = t % G
                slot = t % DEPTH
                k = t // DEPTH
                v.wait_ge(ga_sems[slot], 16 * (k + 1))
                # t = a*cab + cb
                v.tensor_scalar(
                    tt[:], ga[:, slot, 0, :],
                    cf_sb[:, g, 3:4], cf_sb[:, g, 2:3],
                    Alu.mult, Alu.add,
                )
                # r = a*ca + c0
                v.tensor_scalar(
                    rr[:], ga[:, slot, 0, :],
                    cf_sb[:, g, 1:2], cf_sb[:, g, 0:1],
                    Alu.mult, Alu.add,
                )
                v.wait_ge(gb_sems[slot], 16 * (k + 1))
                # u = t*b
                v.tensor_tensor(uu[:], tt[:], gb[:, slot, 0, :], Alu.mult)
                if t >= 2:
                    v.wait_ge(act_sem, t - 1)  # y16 slot WAR vs ACT read
                # y16 = u + r
                v.tensor_tensor(
                    y16[:, t % 2, :], uu[:], rr[:], Alu.add,
                ).then_inc(dve_sem, 1)

        @block.scalar
        def _(act: bass.BassScalarEngine):
            act.wait_ge(setup_sem, 48)
            for t in range(T):
                slot = t % DEPTH
                act.wait_ge(dve_sem, t + 1)
                if t >= DEPTH:
                    act.wait_ge(y_sems[slot], 16 * (t // DEPTH))
                act.activation(
                    y8[:, slot, :], y16[:, t % 2, :], Act.Identity,
                    bias=yc_sb[:, 1:2], scale=yc_sb[:, 0:1],
                ).then_inc(act_sem, 1)

        @block.sync
        def _(sp: bass.BassEngine):
            for t in range(T):
                g = t % G
                slot = t % DEPTH
                sp.wait_ge(act_sem, t + 1)
                sp.dma_start(y[:, g, :], y8[:, slot, :]).then_inc(
                    y_sems[slot], 16)
            for s_i in range(DEPTH):
                sp.wait_ge(y_sems[s_i], 16 * ((T - 1 - s_i) // DEPTH + 1))

    nc.compile()
    return nc


def _prep_host(x, weights, idx_a, idx_b):
    x = np.asarray(x, dtype=np.float32)
    w = np.asarray(weights, dtype=np.float32)
    e = np.exp(w - w.max(axis=1, keepdims=True))
    sm = e / e.sum(axis=1, keepdims=True)
    coeffs = (sm @ GATE_COEFFS).astype(np.float32)          # [8192, 4]
    xt = np.ascontiguousarray(x.T).astype(np.float16)       # [8192, 4096]
    ia = np.asarray(idx_a).astype(np.int16)
    ib = np.asarray(idx_b).astype(np.int16)

    def wrap(seq):  # [n] -> [128, n/16]: j at [j%16, j//16], tiled to 128
        m = seq.reshape(len(seq) // 16, 16).T
        return np.tile(m, (P // 16, 1))

    in_maps = []
    for c in range(N_CORES):
        sl = slice(c * OSHARD, (c + 1) * OSHARD)
        idxw = np.ascontiguousarray(
            np.concatenate([wrap(ia[sl]), wrap(ib[sl])], axis=1))  # [128,128]
        cfc = np.ascontiguousarray(
            coeffs[sl].reshape(G, P, 4).transpose(1, 0, 2))        # [128,8,4]
        ycc = np.tile(np.array([[YSCALE, YBIAS_ENC]], np.float32), (P, 1))
        in_maps.append({"xt": xt, "idxw": idxw, "cf": cfc, "yc": ycc})
    return in_maps


def _in_maps(x, weights, idx_a, idx_b):
    return _prep_host(x, weights, idx_a, idx_b)


def kernel(x, weights, idx_a, idx_b):
    from concourse.bass_utils import run_bass_kernel_spmd

    in_maps = _in_maps(x, weights, idx_a, idx_b)
    if "nc" not in _CACHE:
        _CACHE["nc"] = _build_nc()
    nc = _CACHE["nc"]
    res = run_bass_kernel_spmd(nc, in_maps, list(range(N_CORES)))
    outs = []
    for c in range(N_CORES):
        yc = res.results[c]["y"]                    # [128, G, 4096] uint8
        yf = (yc.astype(np.float32) - YBIAS_DEC) / YSCALE
        outs.append(yf.transpose(2, 1, 0).reshape(BATCH, OSHARD))
    return np.concatenate(outs, axis=1).astype(np.float32)
